# revision 1
# baseline (speedup 1.0000x reference)
"""Embedding-lookup kernel for TRN2 (8 NeuronCores, SPMD data-parallel).

Reference semantics (B=32, S=8192, D=512):
    table = concat(11 per-type tables, unknown_embed)   # [1726, 512] f32
    out[b, s] = table[flat_map[input_ids[b, s]]]

Strategy per core (batch-sharded, 4 rows = 32768 tokens/core):
  1. Concat the 12 table pieces into one DRAM buffer tbl_cat [1726, 512].
  2. dma_gather composes flat_map into the table (tbl_fin[g] = tbl_cat[flat_map[g]])
     so the main loop is a single-level lookup (exact for any flat_map).
     Split into 2x896 indices: the SWDGE ring carveout fits ~65-80
     descriptors per DMA engine and the gather decode reserves
     num_idxs/16+1 up front, so num_idxs > ~1024 hangs the engine.
  3. Main loop: 32 chunks x 1024 tokens. Each chunk is one SWDGE dma_gather
     (HBM table rows -> SBUF) with a token permutation chosen so partition b
     holds 8 *consecutive* tokens; the HWDGE write-back to the output is
     then 128 descriptors x 16 KiB contiguous. 4-way buffering with
     per-buffer semaphores (DMA completions are unordered across
     instructions sharing a semaphore, so each sem tracks at most one
     outstanding transfer).
"""

import numpy as np

import concourse.bass as bass
import concourse.bacc as bacc
import concourse.mybir as mybir
from concourse.bass_utils import run_bass_kernel_spmd
from concourse.library_config import mlp

# ---- problem dims (hardcoded per contract) ----
B, S, D = 32, 8192, 512
NCORES = 8
BPC = B // NCORES            # batch rows per core
T = BPC * S                  # tokens per core = 32768
VOCAB = 1725
VROWS = VOCAB + 1            # fused table rows (incl. unknown)
RIDX = 1792                  # remap gather total idxs (= 14*128), fills dst
RSPLIT = 896                 # per-instruction remap idxs (ring-capacity cap)
CHUNK = 1024                 # tokens per main gather (ring-capacity cap)
NCH = T // CHUNK             # 32 chunks
A = CHUNK // 128             # tokens per partition per chunk = 8
NBUF = 4                     # main-loop buffers

TAB_SPECS = [
    ("special_tab", 3), ("event_tab", 9), ("time_tab", 512), ("note_tab", 128),
    ("vel_tab", 32), ("prog_tab", 129), ("local_tab", 16), ("ccnum_tab", 128),
    ("ccval_tab", 128), ("progval_tab", 128), ("dur_tab", 512),
]

f32 = mybir.dt.float32
i32 = mybir.dt.int32
i16 = mybir.dt.int16


def build_nc(_mode: str = "full", _reps: int = 1, _nq: int = 1, _nbuf: int = None) -> bacc.Bacc:
    global NBUF
    if _nbuf is not None:
        NBUF = _nbuf
    nc = bacc.Bacc("TRN2", target_bir_lowering=False, debug=False,
                   num_swdge_queues=_nq)

    ids = nc.dram_tensor("ids", [T], i32, kind="ExternalInput")
    fmap = nc.dram_tensor("flat_map", [VOCAB], i32, kind="ExternalInput")
    tabs = {name: nc.dram_tensor(name, [n, D], f32, kind="ExternalInput")
            for name, n in TAB_SPECS}
    unk = nc.dram_tensor("unknown_embed", [D], f32, kind="ExternalInput")
    out = nc.dram_tensor("out", [T, D], f32, kind="ExternalOutput")

    tbl_cat = nc.dram_tensor("tbl_cat", [VROWS, D], f32)
    tbl_fin = nc.dram_tensor("tbl_fin", [RIDX, D], f32)

    CC = CHUNK // 16 // A     # inner id-load groups per chunk = 8

    from contextlib import ExitStack
    with ExitStack() as stack:
        ec = stack.enter_context
        fm32 = ec(nc.sbuf_tensor("fm32", [16, RIDX // 16], i32))
        fm16 = ec(nc.sbuf_tensor("fm16", [128, RIDX // 16], i16))
        rdst = ec(nc.sbuf_tensor("rdst", [128, (RIDX // 128) * D], f32))
        ids32 = ec(nc.sbuf_tensor("ids32", [16, T // 16], i32))
        ids16 = ec(nc.sbuf_tensor("ids16", [128, T // 16], i16))
        gbuf = ec(nc.sbuf_tensor("gbuf", [128, NBUF * A * D], f32))
        s_cat = ec(nc.semaphore("s_cat"))    # table concat DMAs
        s_ms = ec(nc.semaphore("s_ms"))      # memset done
        s_fm = ec(nc.semaphore("s_fm"))      # flat_map loads
        s_ids = ec(nc.semaphore("s_ids"))    # ids loads
        s_cast = ec(nc.semaphore("s_cast"))  # DVE casts
        s_rep = ec(nc.semaphore("s_rep"))    # fm16 replicate copies
        s_rep2 = ec(nc.semaphore("s_rep2"))  # ids16 replicate copies
        s_gr = ec(nc.semaphore("s_gr"))      # remap gather completions
        s_tf = ec(nc.semaphore("s_tf"))      # tbl_fin writeback
        s_g = [ec(nc.semaphore(f"s_g{i}")) for i in range(NBUF)]  # per-buffer gathers
        s_w = [ec(nc.semaphore(f"s_w{i}")) for i in range(NBUF)]  # per-buffer writes
        block = ec(nc.Block())

        @block.vector
        def _(v: bass.BassEngine):
            v.memset(fm32[:, :], 0).then_inc(s_ms, 1)
            v.wait_ge(s_fm, 32)
            v.tensor_copy(fm16[0:16, :], fm32[:, :]).then_inc(s_cast, 1)
            v.wait_ge(s_ids, 16 * NCH)
            # cast i32->i16 and permute (cc a) -> (a cc) within each chunk's
            # block, so the gather's wrapped idx order maps partition b to A
            # consecutive tokens.
            for c in range(NCH):
                csl = slice(c * (CHUNK // 16), (c + 1) * (CHUNK // 16))
                v.tensor_copy(
                    ids16[0:16, csl].rearrange("p (a cc) -> p a cc", a=A, cc=CC),
                    ids32[:, csl].rearrange("p (cc a) -> p a cc", a=A, cc=CC),
                ).then_inc(s_cast, 1)

        @block.sync
        def _(s: bass.BassEngine):
            # 1. concat table pieces into tbl_cat (DRAM->DRAM)
            off = 0
            for name, n in TAB_SPECS:
                s.dma_start(tbl_cat[off:off + n, :], tabs[name][:, :]).then_inc(s_cat, 16)
                off += n
            s.dma_start(tbl_cat[VOCAB, :], unk[:]).then_inc(s_cat, 16)

            # 2. flat_map -> wrapped idx layout [p, s] = flat_map[s*16+p]
            s.wait_ge(s_ms, 1)
            with nc.allow_non_contiguous_dma(reason="1.7K-entry one-time idx load"):
                s.dma_start(fm32[0:16, 0:107],
                            fmap[0:1712].rearrange("(s p) -> p s", p=16)).then_inc(s_fm, 16)
                s.dma_start(fm32[0:13, 107:108],
                            fmap[1712:1725].rearrange("(s p) -> p s", p=13)).then_inc(s_fm, 16)

            # 3. ids, contiguous per chunk:
            #    ids32[p, c*(CHUNK//16) + cc*A + a] = ids[c*CHUNK + cc*16*A + p*A + a]
            for c in range(NCH):
                src = ids[c * CHUNK:(c + 1) * CHUNK].rearrange(
                    "(cc p a) -> p cc a", p=16, cc=CC, a=A)
                dst = ids32[:, c * (CHUNK // 16):(c + 1) * (CHUNK // 16)].rearrange(
                    "p (cc a) -> p cc a", a=A, cc=CC)
                s.dma_start(dst, src).then_inc(s_ids, 16)

            # 4. replicate int16 idx tiles to all 8 partition groups
            s.wait_ge(s_cast, 1)
            for k in range(1, 8):
                s.dma_start(fm16[16 * k:16 * (k + 1), :], fm16[0:16, :]).then_inc(s_rep, 16)
            s.wait_ge(s_cast, 1 + NCH)
            for k in range(1, 8):
                s.dma_start(ids16[16 * k:16 * (k + 1), :], ids16[0:16, :]).then_inc(s_rep2, 16)

            # 5. write back the flat_map-composed table
            s.wait_ge(s_gr, 32)
            s.dma_start(tbl_fin[:, :].rearrange("(j p) e -> p j e", p=128),
                        rdst[:, :].rearrange("p (j e) -> p j e", e=D)).then_inc(s_tf, 16)

            # 6. chunk output writes: partition b holds rows b*A..b*A+A-1
            if _mode != "nowrite":
                for m in range(_reps * NCH):
                    c = m % NCH
                    h, r = m % NBUF, m // NBUF
                    if _mode != "nogather":
                        s.wait_ge(s_g[h], 16 * (r + 1))
                    s.dma_start(
                        out[c * CHUNK:(c + 1) * CHUNK, :].rearrange("(b x) e -> b (x e)", x=A),
                        gbuf[:, h * A * D:(h + 1) * A * D],
                    ).then_inc(s_w[h], 16)
                for h in range(NBUF):
                    s.wait_ge(s_w[h], 16 * (_reps * NCH // NBUF))
            else:
                # still write the last buffer once so `out` has a writer
                s.wait_ge(s_g[NBUF - 1], 16 * (NCH // NBUF))
                s.dma_start(
                    out[0:CHUNK, :].rearrange("(b x) e -> b (x e)", x=A),
                    gbuf[:, (NBUF - 1) * A * D:NBUF * A * D],
                ).then_inc(s_w[0], 16)
                s.wait_ge(s_w[0], 16)

        @block.gpsimd
        def _(g: bass.BassGpSimd):
            g.load_library(mlp)
            # remap gathers: tbl_fin row g = tbl_cat[flat_map[g]], split to
            # stay under the ring-capacity cap. Waiting for the sum (32)
            # covers both (an all-complete wait is order-safe).
            g.wait_ge(s_cat, 16 * 12)
            g.wait_ge(s_rep, 16 * 7)
            half = RSPLIT // 16               # idx columns per split
            jh = RSPLIT // 128                # dst slots per split
            for i in range(2):
                g.dma_gather(
                    rdst[:, i * jh * D:(i + 1) * jh * D].rearrange("p (j e) -> p j e", e=D),
                    tbl_cat[:, :], fm16[:, i * half:(i + 1) * half],
                    RSPLIT, RSPLIT, D,
                ).then_inc(s_gr, 16)
            # main gathers
            g.wait_ge(s_tf, 16)
            g.wait_ge(s_rep2, 16 * 7)
            if _mode != "nogather":
                for m in range(_reps * NCH):
                    c = m % NCH
                    h, r = m % NBUF, m // NBUF
                    if m >= NBUF and _mode != "nowrite":
                        g.wait_ge(s_w[h], 16 * r)
                    if _mode == "nowrite" and m >= 2 * NBUF:
                        # self-throttle so gathers don't all queue at once
                        g.wait_ge(s_g[(m - 2 * NBUF) % NBUF], 16 * ((m - 2 * NBUF) // NBUF + 1))
                    g.dma_gather(
                        gbuf[:, h * A * D:(h + 1) * A * D].rearrange("p (n e) -> p n e", e=D),
                        tbl_fin[:, :],
                        ids16[:, c * (CHUNK // 16):(c + 1) * (CHUNK // 16)],
                        CHUNK, CHUNK, D,
                        queue_num=m % _nq,
                    ).then_inc(s_g[h], 16)
            else:
                for h in range(NBUF):
                    g.dma_gather(
                        gbuf[:, h * A * D:(h + 1) * A * D].rearrange("p (n e) -> p n e", e=D),
                        tbl_fin[:, :],
                        ids16[:, 0:CHUNK // 16],
                        CHUNK, CHUNK, D,
                    ).then_inc(s_g[h], 16)

    nc.compile()
    return nc


_NC_CACHE: list = [None]


def _get_nc() -> bacc.Bacc:
    if _NC_CACHE[0] is None:
        _NC_CACHE[0] = build_nc()
    return _NC_CACHE[0]


def make_in_maps(**inputs) -> list[dict]:
    ids_full = np.ascontiguousarray(np.asarray(inputs["input_ids"], dtype=np.int32))
    shared = {
        "flat_map": np.ascontiguousarray(np.asarray(inputs["flat_map"], dtype=np.int32)),
        "unknown_embed": np.ascontiguousarray(
            np.asarray(inputs["unknown_embed"], dtype=np.float32)),
    }
    for name, n in TAB_SPECS:
        shared[name] = np.ascontiguousarray(np.asarray(inputs[name], dtype=np.float32))
    in_maps = []
    for c in range(NCORES):
        m = dict(shared)
        m["ids"] = ids_full[c * BPC:(c + 1) * BPC, :].reshape(-1).copy()
        in_maps.append(m)
    return in_maps


def kernel(**inputs) -> np.ndarray:
    nc = _get_nc()
    in_maps = make_in_maps(**inputs)
    res = run_bass_kernel_spmd(nc, in_maps, list(range(NCORES)))
    outs = [res.results[c]["out"] for c in range(NCORES)]
    return np.concatenate(outs, axis=0).reshape(B, S, D)


def kernel_traced(**inputs):
    """Like kernel() but with NTFF profiling; returns (output, BassKernelResults)."""
    nc = _get_nc()
    in_maps = make_in_maps(**inputs)
    res = run_bass_kernel_spmd(nc, in_maps, list(range(NCORES)), trace=True)
    outs = [res.results[c]["out"] for c in range(NCORES)]
    return np.concatenate(outs, axis=0).reshape(B, S, D), res



# revision 4
# speedup vs baseline: 1.1332x; 1.1332x over previous
"""Embedding-lookup kernel for TRN2 (8 NeuronCores, SPMD data-parallel).

Reference semantics (B=32, S=8192, D=512):
    table = concat(11 per-type tables, unknown_embed)   # [1726, 512] f32
    out[b, s] = table[flat_map[input_ids[b, s]]]

Strategy per core (batch-sharded, 4 rows = 32768 tokens/core). The DMA
fabric (16 engines, ~405 GB/s aggregate measured) is the bottleneck, so
the main loop gathers the table at HALF width (f16) and upconverts to
f32 on the otherwise-idle DVE + Activation engines before the output
writes:

  1. Host pre-concats the 12 table pieces into one tbl_cat input
     [1726, 512] f32 (input marshalling, like the ids reshape).
  2. dma_gather composes flat_map into the table
     (rdst[g] = tbl_cat[flat_map[g]], exact for any flat_map), split
     2x896 indices to fit the SWDGE ring carveout (~1024 descs).
  3. DVE casts the composed table f32 -> f16; writeback to DRAM
     tbl_fin16 [1792, 512] f16.
  4. Main loop: 32 chunks x 1024 tokens. Each chunk is one SWDGE
     dma_gather (f16 rows -> SBUF, 1 KiB descriptors) with a token
     permutation so partition b holds 8 *consecutive* tokens; DVE (even
     chunks) or Act (odd chunks) upconverts f16 -> f32; the HWDGE
     write-back is then 128 descriptors x 16 KiB contiguous. 4-way
     buffering with per-buffer semaphores (DMA completions are
     unordered across instructions sharing a semaphore).

The ids wrap (cc p a) is uniform across chunks, so the whole ids load
is ONE 3-dim DMA and the whole i32->i16 permute-cast ONE 4-dim DVE
copy. ids load rides the Activation engine so the sync engine reaches
the fm replicates (remap-gather dependency) immediately.

Error budget: f16 table rounding gives rel err <= 2^-11 of the
0.02-scale values, ~40x under the 2e-2 relative gate.
"""

import numpy as np

import concourse.bass as bass
import concourse.bacc as bacc
import concourse.mybir as mybir
from concourse.bass_utils import run_bass_kernel_spmd
from concourse.library_config import mlp

# ---- problem dims (hardcoded per contract) ----
B, S, D = 32, 8192, 512
NCORES = 8
BPC = B // NCORES            # batch rows per core
T = BPC * S                  # tokens per core = 32768
VOCAB = 1725
VROWS = VOCAB + 1            # fused table rows (incl. unknown)
RIDX = 1792                  # remap gather total idxs (= 14*128), fills dst
RSPLIT = 896                 # per-instruction remap idxs (ring-capacity cap)
CHUNK = 1024                 # tokens per main gather (ring-capacity cap)
NCH = T // CHUNK             # 32 chunks
A = CHUNK // 128             # tokens per partition per chunk = 8
NBUF = 4                     # main-loop buffers

f32 = mybir.dt.float32
f16 = mybir.dt.float16
i32 = mybir.dt.int32
i16 = mybir.dt.int16


def build_nc() -> bacc.Bacc:
    nc = bacc.Bacc("TRN2", target_bir_lowering=False, debug=False)

    ids = nc.dram_tensor("ids", [T], i32, kind="ExternalInput")
    fmap = nc.dram_tensor("flat_map", [VOCAB], i32, kind="ExternalInput")
    tbl_cat = nc.dram_tensor("tbl_cat", [VROWS, D], f32, kind="ExternalInput")
    out = nc.dram_tensor("out", [T, D], f32, kind="ExternalOutput")

    tbl_fin16 = nc.dram_tensor("tbl_fin16", [RIDX, D], f16)

    CC = CHUNK // 16 // A     # inner id wrap groups per chunk = 8
    X = NCH * CC              # global id wrap groups = 256

    from contextlib import ExitStack
    with ExitStack() as stack:
        ec = stack.enter_context
        fm32 = ec(nc.sbuf_tensor("fm32", [16, RIDX // 16], i32))
        fm16 = ec(nc.sbuf_tensor("fm16", [128, RIDX // 16], i16))
        rdst = ec(nc.sbuf_tensor("rdst", [128, (RIDX // 128) * D], f32))
        rq16 = ec(nc.sbuf_tensor("rq16", [128, (RIDX // 128) * D], f16))
        ids32 = ec(nc.sbuf_tensor("ids32", [16, T // 16], i32))
        ids16 = ec(nc.sbuf_tensor("ids16", [128, T // 16], i16))
        g16 = ec(nc.sbuf_tensor("g16", [128, NBUF * A * D], f16))
        g32 = ec(nc.sbuf_tensor("g32", [128, NBUF * A * D], f32))
        s_ms = ec(nc.semaphore("s_ms"))      # memset done
        s_fm = ec(nc.semaphore("s_fm"))      # flat_map loads
        s_ids = ec(nc.semaphore("s_ids"))    # ids load
        s_cast = ec(nc.semaphore("s_cast"))  # DVE idx casts (fm, ids)
        s_rep = ec(nc.semaphore("s_rep"))    # fm16 replicate copies
        s_rep2 = ec(nc.semaphore("s_rep2"))  # ids16 replicate copies
        s_gr = ec(nc.semaphore("s_gr"))      # remap gather completions
        s_q = ec(nc.semaphore("s_q"))        # rdst -> rq16 cast
        s_tf = ec(nc.semaphore("s_tf"))      # tbl_fin16 writeback
        s_g = [ec(nc.semaphore(f"s_g{i}")) for i in range(NBUF)]  # gathers
        s_c = [ec(nc.semaphore(f"s_c{i}")) for i in range(NBUF)]  # upcasts
        s_w = [ec(nc.semaphore(f"s_w{i}")) for i in range(NBUF)]  # writes
        block = ec(nc.Block())

        @block.vector
        def _(v: bass.BassEngine):
            v.memset(fm32[:, :], 0).then_inc(s_ms, 1)
            v.wait_ge(s_fm, 32)
            v.tensor_copy(fm16[0:16, :], fm32[:, :]).then_inc(s_cast, 1)
            # cast i32->i16 and permute (cc a) -> (a cc) within each chunk's
            # block, so the gather's wrapped idx order maps partition b to A
            # consecutive tokens. One fused 4-dim copy over all chunks.
            v.wait_ge(s_ids, 16)
            v.tensor_copy(
                ids16[0:16, :].rearrange("p (c a cc) -> p c a cc", c=NCH, a=A, cc=CC),
                ids32[:, :].rearrange("p (c cc a) -> p c a cc", c=NCH, a=A, cc=CC),
            ).then_inc(s_cast, 1)
            # composed-table downcast f32 -> f16
            v.wait_ge(s_gr, 32)
            v.tensor_copy(rq16[:, :], rdst[:, :]).then_inc(s_q, 1)
            # upconvert even chunks f16 -> f32
            for m in range(0, NCH, 2):
                h, r = m % NBUF, m // NBUF
                v.wait_ge(s_g[h], 16 * (r + 1))
                v.tensor_copy(
                    g32[:, h * A * D:(h + 1) * A * D],
                    g16[:, h * A * D:(h + 1) * A * D],
                ).then_inc(s_c[h], 1)

        @block.scalar
        def _(sc: bass.BassEngine):
            # ids, one 3-dim DMA: ids32[p, x*A + a] = ids[x*128 + p*A + a]
            sc.dma_start(
                ids32[:, :].rearrange("p (x a) -> p x a", x=X, a=A),
                ids[:].rearrange("(x p a) -> p x a", x=X, p=16, a=A),
            ).then_inc(s_ids, 16)
            # upconvert odd chunks f16 -> f32
            for m in range(1, NCH, 2):
                h, r = m % NBUF, m // NBUF
                sc.wait_ge(s_g[h], 16 * (r + 1))
                sc.activation(
                    g32[:, h * A * D:(h + 1) * A * D],
                    g16[:, h * A * D:(h + 1) * A * D],
                    mybir.ActivationFunctionType.Copy,
                ).then_inc(s_c[h], 1)

        @block.sync
        def _(s: bass.BassEngine):
            # 1. flat_map -> wrapped idx layout [p, s] = flat_map[s*16+p]
            s.wait_ge(s_ms, 1)
            with nc.allow_non_contiguous_dma(reason="1.7K-entry one-time idx load"):
                s.dma_start(fm32[0:16, 0:107],
                            fmap[0:1712].rearrange("(s p) -> p s", p=16)).then_inc(s_fm, 16)
                s.dma_start(fm32[0:13, 107:108],
                            fmap[1712:1725].rearrange("(s p) -> p s", p=13)).then_inc(s_fm, 16)

            # 2. replicate int16 idx tiles to all 8 partition groups
            s.wait_ge(s_cast, 1)
            for k in range(1, 8):
                s.dma_start(fm16[16 * k:16 * (k + 1), :], fm16[0:16, :]).then_inc(s_rep, 16)
            s.wait_ge(s_cast, 2)
            for k in range(1, 8):
                s.dma_start(ids16[16 * k:16 * (k + 1), :], ids16[0:16, :]).then_inc(s_rep2, 16)

            # 3. write back the flat_map-composed f16 table
            s.wait_ge(s_q, 1)
            s.dma_start(tbl_fin16[:, :].rearrange("(j p) e -> p j e", p=128),
                        rq16[:, :].rearrange("p (j e) -> p j e", e=D)).then_inc(s_tf, 16)

            # 4. chunk output writes: partition b holds rows b*A..b*A+A-1
            for m in range(NCH):
                h, r = m % NBUF, m // NBUF
                s.wait_ge(s_c[h], r + 1)
                s.dma_start(
                    out[m * CHUNK:(m + 1) * CHUNK, :].rearrange("(b x) e -> b (x e)", x=A),
                    g32[:, h * A * D:(h + 1) * A * D],
                ).then_inc(s_w[h], 16)
            for h in range(NBUF):
                s.wait_ge(s_w[h], 16 * (NCH // NBUF))

        @block.gpsimd
        def _(g: bass.BassGpSimd):
            g.load_library(mlp)
            # remap gathers: rdst row g = tbl_cat[flat_map[g]], split to
            # stay under the ring-capacity cap. Waiting for the sum (32)
            # covers both (an all-complete wait is order-safe).
            g.wait_ge(s_rep, 16 * 7)
            half = RSPLIT // 16               # idx columns per split
            jh = RSPLIT // 128                # dst slots per split
            for i in range(2):
                g.dma_gather(
                    rdst[:, i * jh * D:(i + 1) * jh * D].rearrange("p (j e) -> p j e", e=D),
                    tbl_cat[:, :], fm16[:, i * half:(i + 1) * half],
                    RSPLIT, RSPLIT, D,
                ).then_inc(s_gr, 16)
            # main gathers (f16 rows, 1 KiB descriptors)
            g.wait_ge(s_tf, 16)
            g.wait_ge(s_rep2, 16 * 7)
            for m in range(NCH):
                h, r = m % NBUF, m // NBUF
                if m >= NBUF:
                    g.wait_ge(s_w[h], 16 * r)
                g.dma_gather(
                    g16[:, h * A * D:(h + 1) * A * D].rearrange("p (n e) -> p n e", e=D),
                    tbl_fin16[:, :],
                    ids16[:, m * (CHUNK // 16):(m + 1) * (CHUNK // 16)],
                    CHUNK, CHUNK, D,
                ).then_inc(s_g[h], 16)

    nc.compile()
    return nc


_NC_CACHE: list = [None]


def _get_nc() -> bacc.Bacc:
    if _NC_CACHE[0] is None:
        _NC_CACHE[0] = build_nc()
    return _NC_CACHE[0]


TAB_ORDER = [
    "special_tab", "event_tab", "time_tab", "note_tab", "vel_tab", "prog_tab",
    "local_tab", "ccnum_tab", "ccval_tab", "progval_tab", "dur_tab",
]


def make_in_maps(**inputs) -> list[dict]:
    ids_full = np.ascontiguousarray(np.asarray(inputs["input_ids"], dtype=np.int32))
    tbl_cat = np.concatenate(
        [np.asarray(inputs[name], dtype=np.float32) for name in TAB_ORDER]
        + [np.asarray(inputs["unknown_embed"], dtype=np.float32)[None, :]],
        axis=0)
    shared = {
        "flat_map": np.ascontiguousarray(np.asarray(inputs["flat_map"], dtype=np.int32)),
        "tbl_cat": np.ascontiguousarray(tbl_cat),
    }
    in_maps = []
    for c in range(NCORES):
        m = dict(shared)
        m["ids"] = ids_full[c * BPC:(c + 1) * BPC, :].reshape(-1).copy()
        in_maps.append(m)
    return in_maps


def kernel(**inputs) -> np.ndarray:
    nc = _get_nc()
    in_maps = make_in_maps(**inputs)
    res = run_bass_kernel_spmd(nc, in_maps, list(range(NCORES)))
    outs = [res.results[c]["out"] for c in range(NCORES)]
    return np.concatenate(outs, axis=0).reshape(B, S, D)


def kernel_traced(**inputs):
    """Like kernel() but with NTFF profiling; returns (output, BassKernelResults)."""
    nc = _get_nc()
    in_maps = make_in_maps(**inputs)
    res = run_bass_kernel_spmd(nc, in_maps, list(range(NCORES)), trace=True)
    outs = [res.results[c]["out"] for c in range(NCORES)]
    return np.concatenate(outs, axis=0).reshape(B, S, D), res


# revision 7
# speedup vs baseline: 1.2759x; 1.1259x over previous
"""Embedding-lookup kernel for TRN2 (8 NeuronCores, SPMD data-parallel).

Reference semantics (B=32, S=8192, D=512):
    table = concat(11 per-type tables, unknown_embed)   # [1726, 512] f32
    out[b, s] = table[flat_map[input_ids[b, s]]]

Strategy per core (batch-sharded, 4 rows = 32768 tokens/core). Two
hardware limits drive the design (both measured via NTFF traces):
  - DMA fabric: 16 engines, ~405 GB/s aggregate.
  - SWDGE dma_gather descriptor generation on the Q7/Pool engine costs
    ~9.4 ns/descriptor (= per gathered row), which paces the gather
    stream at ~32768 x 9.4ns ~= 308 us regardless of row byte width.
So the main loop gathers the table at HALF width (f16, halving fabric
read traffic so output writes overlap fully) and upconverts to f32 on
the otherwise-idle DVE + Activation engines before the HWDGE writes.

Pipeline per core:
  1. Host pre-concats the 12 table pieces into one tbl_cat input
     [1726, 512] f32, and pre-wraps flat_map/ids into the int16
     16-partition-wrapped, 8x-replicated SBUF layouts the gather ucode
     consumes (pure index marshalling; values unchanged).
  2. Two pipelined SWDGE dma_gathers compose flat_map into the table
     (rdst[g] = tbl_cat[flat_map[g]], exact for any flat_map), each
     slice DVE-cast f32->f16 and written back to DRAM tbl_fin16
     [1792, 512] f16 while the next slice gathers.
  3. Main loop: 32 chunks x 1024 tokens. Each chunk is one SWDGE
     dma_gather (f16 rows -> SBUF, 1 KiB descriptors) whose wrapped idx
     order maps partition b to 8 *consecutive* tokens; DVE (even
     chunks) or Act (odd chunks) upconverts f16 -> f32; the HWDGE
     write-back is then 128 descriptors x 16 KiB contiguous. 4-way
     buffering with per-buffer semaphores (DMA completions are
     unordered across instructions sharing a semaphore).

Error budget: f16 table rounding gives rel err <= 2^-11 of the
0.02-scale values, ~40x under the 2e-2 relative gate.
"""

import numpy as np

import concourse.bass as bass
import concourse.bacc as bacc
import concourse.mybir as mybir
from concourse.bass_utils import run_bass_kernel_spmd
from concourse.library_config import mlp

# ---- problem dims (hardcoded per contract) ----
B, S, D = 32, 8192, 512
NCORES = 8
BPC = B // NCORES            # batch rows per core
T = BPC * S                  # tokens per core = 32768
VOCAB = 1725
VROWS = VOCAB + 1            # fused table rows (incl. unknown)
RIDX = 1792                  # composed-table rows incl. padding (= 14*128)
RSPLIT = 896                 # per-instruction remap idxs (ring-capacity cap)
CHUNK = 1024                 # tokens per main gather (ring-capacity cap)
NCH = T // CHUNK             # 32 chunks
A = CHUNK // 128             # tokens per partition per chunk = 8
CC = CHUNK // 16 // A        # idx wrap groups per chunk = 8
NBUF = 4                     # main-loop buffers
NQ = 4                       # SWDGE queues (desc-gen runs ahead of ring reclaim)

f32 = mybir.dt.float32
f16 = mybir.dt.float16
i16 = mybir.dt.int16


def build_nc(_nq: int = NQ) -> bacc.Bacc:
    nc = bacc.Bacc("TRN2", target_bir_lowering=False, debug=False,
                   num_swdge_queues=_nq)

    ids16d = nc.dram_tensor("ids16w", [128, T // 16], i16, kind="ExternalInput")
    fm16d = nc.dram_tensor("fm16w", [128, RIDX // 16], i16, kind="ExternalInput")
    tbl_cat = nc.dram_tensor("tbl_cat", [VROWS, D], f32, kind="ExternalInput")
    out = nc.dram_tensor("out", [T, D], f32, kind="ExternalOutput")

    tbl_fin16 = nc.dram_tensor("tbl_fin16", [RIDX, D], f16)

    JH = RSPLIT // 128            # composed slots per remap slice = 7
    HALF = RSPLIT // 16           # idx columns per remap slice = 56

    from contextlib import ExitStack
    with ExitStack() as stack:
        ec = stack.enter_context
        fm16 = ec(nc.sbuf_tensor("fm16", [128, RIDX // 16], i16))
        rdst = ec(nc.sbuf_tensor("rdst", [128, (RIDX // 128) * D], f32))
        rq16 = ec(nc.sbuf_tensor("rq16", [128, (RIDX // 128) * D], f16))
        ids16 = ec(nc.sbuf_tensor("ids16", [128, T // 16], i16))
        g16 = ec(nc.sbuf_tensor("g16", [128, NBUF * A * D], f16))
        g32 = ec(nc.sbuf_tensor("g32", [128, NBUF * A * D], f32))
        s_fm = ec(nc.semaphore("s_fm"))      # flat_map load
        s_ids = ec(nc.semaphore("s_ids"))    # ids load
        s_gr = [ec(nc.semaphore(f"s_gr{i}")) for i in range(2)]  # remap slices
        s_q = ec(nc.semaphore("s_q"))        # rdst -> rq16 casts
        s_tf = ec(nc.semaphore("s_tf"))      # tbl_fin16 writebacks
        s_g = [ec(nc.semaphore(f"s_g{i}")) for i in range(NBUF)]  # gathers
        s_c = [ec(nc.semaphore(f"s_c{i}")) for i in range(NBUF)]  # upcasts
        s_w = [ec(nc.semaphore(f"s_w{i}")) for i in range(NBUF)]  # writes
        block = ec(nc.Block())

        @block.vector
        def _(v: bass.BassEngine):
            # composed-table downcast f32 -> f16, per remap slice
            for i in range(2):
                v.wait_ge(s_gr[i], 16)
                v.tensor_copy(rq16[:, i * JH * D:(i + 1) * JH * D],
                              rdst[:, i * JH * D:(i + 1) * JH * D]).then_inc(s_q, 1)
            # upconvert even chunks f16 -> f32
            for m in range(0, NCH, 2):
                h, r = m % NBUF, m // NBUF
                v.wait_ge(s_g[h], 16 * (r + 1))
                v.tensor_copy(
                    g32[:, h * A * D:(h + 1) * A * D],
                    g16[:, h * A * D:(h + 1) * A * D],
                ).then_inc(s_c[h], 1)

        @block.scalar
        def _(sc: bass.BassEngine):
            # ids, one contiguous DMA into the wrapped+replicated layout
            sc.dma_start(ids16[:, :], ids16d[:, :]).then_inc(s_ids, 16)
            # upconvert odd chunks f16 -> f32
            for m in range(1, NCH, 2):
                h, r = m % NBUF, m // NBUF
                sc.wait_ge(s_g[h], 16 * (r + 1))
                sc.activation(
                    g32[:, h * A * D:(h + 1) * A * D],
                    g16[:, h * A * D:(h + 1) * A * D],
                    mybir.ActivationFunctionType.Copy,
                ).then_inc(s_c[h], 1)

        @block.sync
        def _(s: bass.BassEngine):
            s.dma_start(fm16[:, :], fm16d[:, :]).then_inc(s_fm, 16)

            # write back the flat_map-composed f16 table, per slice
            for i in range(2):
                s.wait_ge(s_q, i + 1)
                s.dma_start(
                    tbl_fin16[i * RSPLIT:(i + 1) * RSPLIT, :].rearrange(
                        "(j p) e -> p j e", p=128),
                    rq16[:, i * JH * D:(i + 1) * JH * D].rearrange(
                        "p (j e) -> p j e", e=D),
                ).then_inc(s_tf, 16)

            # chunk output writes: partition b holds rows b*A..b*A+A-1
            for m in range(NCH):
                h, r = m % NBUF, m // NBUF
                s.wait_ge(s_c[h], r + 1)
                s.dma_start(
                    out[m * CHUNK:(m + 1) * CHUNK, :].rearrange("(b x) e -> b (x e)", x=A),
                    g32[:, h * A * D:(h + 1) * A * D],
                ).then_inc(s_w[h], 16)
            for h in range(NBUF):
                s.wait_ge(s_w[h], 16 * (NCH // NBUF))

        @block.gpsimd
        def _(g: bass.BassGpSimd):
            g.load_library(mlp)
            # remap slices: tbl_fin16 row g = f16(tbl_cat[flat_map[g]])
            g.wait_ge(s_fm, 16)
            for i in range(2):
                g.dma_gather(
                    rdst[:, i * JH * D:(i + 1) * JH * D].rearrange("p (j e) -> p j e", e=D),
                    tbl_cat[:, :], fm16[:, i * HALF:(i + 1) * HALF],
                    RSPLIT, RSPLIT, D,
                ).then_inc(s_gr[i], 16)
            # main gathers (f16 rows, 1 KiB descriptors)
            g.wait_ge(s_tf, 32)
            g.wait_ge(s_ids, 16)
            for m in range(NCH):
                h, r = m % NBUF, m // NBUF
                if m >= NBUF:
                    g.wait_ge(s_w[h], 16 * r)
                g.dma_gather(
                    g16[:, h * A * D:(h + 1) * A * D].rearrange("p (n e) -> p n e", e=D),
                    tbl_fin16[:, :],
                    ids16[:, m * (CHUNK // 16):(m + 1) * (CHUNK // 16)],
                    CHUNK, CHUNK, D,
                    queue_num=m % _nq,
                ).then_inc(s_g[h], 16)

    nc.compile()
    return nc


_NC_CACHE: list = [None]


def _get_nc() -> bacc.Bacc:
    if _NC_CACHE[0] is None:
        _NC_CACHE[0] = build_nc()
    return _NC_CACHE[0]


TAB_ORDER = [
    "special_tab", "event_tab", "time_tab", "note_tab", "vel_tab", "prog_tab",
    "local_tab", "ccnum_tab", "ccval_tab", "progval_tab", "dur_tab",
]


def make_in_maps(**inputs) -> list[dict]:
    ids_full = np.asarray(inputs["input_ids"], dtype=np.int32)
    tbl_cat = np.concatenate(
        [np.asarray(inputs[name], dtype=np.float32) for name in TAB_ORDER]
        + [np.asarray(inputs["unknown_embed"], dtype=np.float32)[None, :]],
        axis=0)
    # flat_map, padded to RIDX, wrapped [q, s] = fm[s*16+q], replicated x8
    fmp = np.zeros(RIDX, dtype=np.int16)
    fmp[:VOCAB] = np.asarray(inputs["flat_map"], dtype=np.int32).astype(np.int16)
    fm16w = np.ascontiguousarray(np.tile(fmp.reshape(-1, 16).T, (8, 1)))
    shared = {
        "fm16w": fm16w,
        "tbl_cat": np.ascontiguousarray(tbl_cat),
    }
    in_maps = []
    for c in range(NCORES):
        ids_c = ids_full[c * BPC:(c + 1) * BPC, :].reshape(-1)
        # wrapped idx layout: idsw[q, c*64 + a*8 + cc] = ids[c*1024+cc*128+q*8+a]
        idsw = ids_c.reshape(NCH, CC, 16, A).transpose(2, 0, 3, 1).reshape(16, -1)
        m = dict(shared)
        m["ids16w"] = np.ascontiguousarray(np.tile(idsw.astype(np.int16), (8, 1)))
        in_maps.append(m)
    return in_maps


def kernel(**inputs) -> np.ndarray:
    nc = _get_nc()
    in_maps = make_in_maps(**inputs)
    res = run_bass_kernel_spmd(nc, in_maps, list(range(NCORES)))
    outs = [res.results[c]["out"] for c in range(NCORES)]
    return np.concatenate(outs, axis=0).reshape(B, S, D)


def kernel_traced(**inputs):
    """Like kernel() but with NTFF profiling; returns (output, BassKernelResults)."""
    nc = _get_nc()
    in_maps = make_in_maps(**inputs)
    res = run_bass_kernel_spmd(nc, in_maps, list(range(NCORES)), trace=True)
    outs = [res.results[c]["out"] for c in range(NCORES)]
    return np.concatenate(outs, axis=0).reshape(B, S, D), res


# revision 9
# speedup vs baseline: 1.3900x; 1.0894x over previous
"""Embedding-lookup kernel for TRN2 (8 NeuronCores, SPMD data-parallel).

Reference semantics (B=32, S=8192, D=512):
    table = concat(11 per-type tables, unknown_embed)   # [1726, 512] f32
    out[b, s] = table[flat_map[input_ids[b, s]]]

Strategy per core (batch-sharded, 4 rows = 32768 tokens/core). Measured
hardware limits that drive the design (NTFF traces):
  - DMA fabric: 16 engines, ~405 GB/s aggregate; output writes (64 MiB
    f32/core) are irreducible, so the gather stream must stay under
    ~1/2 of the fabric for full overlap -> gather the table at f16 and
    upconvert to f32 on the otherwise-idle DVE + Activation engines.
  - SWDGE dma_gather desc-gen on the Q7 engine: ~8.6 ns/row with one
    queue most of it ring-reclaim backpressure; spreading consecutive
    gathers over all 4 SWDGE queues overlaps gen with transfers
    (~5.4 ns/row effective).

Pipeline per core:
  1. Host pre-concats the 12 table pieces into one tbl_cat input
     [1726, 512] f32, and pre-wraps flat_map/ids into the int16
     16-partition-wrapped, 8x-replicated layout the gather ucode
     consumes (pure index marshalling; values unchanged).
  2. Two pipelined SWDGE dma_gathers compose flat_map into the table
     (rdst[g] = tbl_cat[flat_map[g]], exact for any flat_map), each
     slice DVE-cast f32->f16 and written back to DRAM tbl_fin16
     [1792, 512] f16 while the next slice gathers.
  3. Main loop: 32 chunks x 1024 tokens, wrapped idx order mapping
     partition b to 8 *consecutive* tokens, so each HWDGE output write
     is 128 descriptors x 16 KiB contiguous.
     - Chunks 0..K_F32-1 gather f32 rows straight from tbl_cat while
       the f16 table is still being built (their idx columns hold
       host-composed flat_map[ids]; identity flat_map -> the raw ids).
     - Chunks K_F32.. gather f16 rows from tbl_fin16; DVE (even) / Act
       (odd) upconvert to f32.
     Decoupled buffer rings (8x f16 gather bufs, 4x f32 write bufs)
     with per-buffer semaphores (DMA completions are unordered across
     instructions sharing a semaphore).

Error budget: f16 table rounding gives rel err <= 2^-11 of the
0.02-scale values, ~40x under the 2e-2 relative gate.
"""

import numpy as np

import concourse.bass as bass
import concourse.bacc as bacc
import concourse.mybir as mybir
from concourse.bass_utils import run_bass_kernel_spmd
from concourse.library_config import mlp

# ---- problem dims (hardcoded per contract) ----
B, S, D = 32, 8192, 512
NCORES = 8
BPC = B // NCORES            # batch rows per core
T = BPC * S                  # tokens per core = 32768
VOCAB = 1725
VROWS = VOCAB + 1            # fused table rows (incl. unknown)
RIDX = 1792                  # composed-table rows incl. padding (= 14*128)
RSPLIT = 896                 # per-instruction remap idxs (ring-capacity cap)
CHUNK = 1024                 # tokens per main gather (ring-capacity cap)
NCH = T // CHUNK             # 32 chunks
A = CHUNK // 128             # tokens per partition per chunk = 8
CC = CHUNK // 16 // A        # idx wrap groups per chunk = 8
H16 = 8                      # f16 gather buffers
H32 = 4                      # f32 write buffers
NQ = 4                       # SWDGE queues (desc-gen runs ahead of reclaim)
K_F32 = 0                    # leading chunks gathered at f32 from tbl_cat
                             # (>0 trades fabric bytes for prologue overlap;
                             #  needs the writebacks off the sync engine to
                             #  avoid a cross-engine wait cycle — keep 0)

f32 = mybir.dt.float32
f16 = mybir.dt.float16
i16 = mybir.dt.int16


def build_nc(_nq: int = NQ, _k: int = K_F32) -> bacc.Bacc:
    nc = bacc.Bacc("TRN2", target_bir_lowering=False, debug=False,
                   num_swdge_queues=_nq)

    ids16d = nc.dram_tensor("ids16w", [128, T // 16], i16, kind="ExternalInput")
    fm16d = nc.dram_tensor("fm16w", [128, RIDX // 16], i16, kind="ExternalInput")
    tbl_cat = nc.dram_tensor("tbl_cat", [VROWS, D], f32, kind="ExternalInput")
    out = nc.dram_tensor("out", [T, D], f32, kind="ExternalOutput")

    tbl_fin16 = nc.dram_tensor("tbl_fin16", [RIDX, D], f16)

    JH = RSPLIT // 128            # composed slots per remap slice = 7
    HALF = RSPLIT // 16           # idx columns per remap slice = 56

    def g16s(m):                  # f16 gather buffer slice for chunk m
        h = (m - _k) % H16
        return slice(h * A * D, (h + 1) * A * D)

    def g32s(m):                  # f32 write buffer slice for chunk m
        h = m % H32
        return slice(h * A * D, (h + 1) * A * D)

    def idxs(m):                  # wrapped idx columns for chunk m
        return slice(m * (CHUNK // 16), (m + 1) * (CHUNK // 16))

    from contextlib import ExitStack
    with ExitStack() as stack:
        ec = stack.enter_context
        fm16 = ec(nc.sbuf_tensor("fm16", [128, RIDX // 16], i16))
        rdst = ec(nc.sbuf_tensor("rdst", [128, (RIDX // 128) * D], f32))
        rq16 = ec(nc.sbuf_tensor("rq16", [128, (RIDX // 128) * D], f16))
        ids16 = ec(nc.sbuf_tensor("ids16", [128, T // 16], i16))
        g16 = ec(nc.sbuf_tensor("g16", [128, H16 * A * D], f16))
        g32 = ec(nc.sbuf_tensor("g32", [128, H32 * A * D], f32))
        s_fm = ec(nc.semaphore("s_fm"))      # flat_map load
        s_ids = ec(nc.semaphore("s_ids"))    # ids load
        s_gr = [ec(nc.semaphore(f"s_gr{i}")) for i in range(2)]  # remap slices
        s_q = ec(nc.semaphore("s_q"))        # rdst -> rq16 casts
        s_tf = ec(nc.semaphore("s_tf"))      # tbl_fin16 writebacks
        s_g16 = [ec(nc.semaphore(f"s_g16_{i}")) for i in range(H16)]  # f16 gathers
        s_ct = [ec(nc.semaphore(f"s_ct{i}")) for i in range(H16)]     # upcasts
        s_gf = [ec(nc.semaphore(f"s_gf{i}")) for i in range(H32)]     # f32 gathers
        s_w = [ec(nc.semaphore(f"s_w{i}")) for i in range(H32)]       # writes
        block = ec(nc.Block())

        @block.vector
        def _(v: bass.BassEngine):
            # composed-table downcast f32 -> f16, per remap slice
            for i in range(2):
                v.wait_ge(s_gr[i], 16)
                v.tensor_copy(rq16[:, i * JH * D:(i + 1) * JH * D],
                              rdst[:, i * JH * D:(i + 1) * JH * D]).then_inc(s_q, 1)
            # upconvert even f16 chunks
            for m in range(_k, NCH):
                if m % 2:
                    continue
                v.wait_ge(s_g16[(m - _k) % H16], 16 * ((m - _k) // H16 + 1))
                if m >= H32:
                    v.wait_ge(s_w[m % H32], 16 * (m // H32))
                v.tensor_copy(g32[:, g32s(m)], g16[:, g16s(m)]).then_inc(
                    s_ct[(m - _k) % H16], 1)

        @block.scalar
        def _(sc: bass.BassEngine):
            # ids, one contiguous DMA into the wrapped+replicated layout
            sc.dma_start(ids16[:, :], ids16d[:, :]).then_inc(s_ids, 16)
            # upconvert odd f16 chunks
            for m in range(_k, NCH):
                if not m % 2:
                    continue
                sc.wait_ge(s_g16[(m - _k) % H16], 16 * ((m - _k) // H16 + 1))
                if m >= H32:
                    sc.wait_ge(s_w[m % H32], 16 * (m // H32))
                sc.activation(g32[:, g32s(m)], g16[:, g16s(m)],
                              mybir.ActivationFunctionType.Copy).then_inc(
                    s_ct[(m - _k) % H16], 1)

        @block.sync
        def _(s: bass.BassEngine):
            s.dma_start(fm16[:, :], fm16d[:, :]).then_inc(s_fm, 16)

            # write back the flat_map-composed f16 table, per slice
            for i in range(2):
                s.wait_ge(s_q, i + 1)
                s.dma_start(
                    tbl_fin16[i * RSPLIT:(i + 1) * RSPLIT, :].rearrange(
                        "(j p) e -> p j e", p=128),
                    rq16[:, i * JH * D:(i + 1) * JH * D].rearrange(
                        "p (j e) -> p j e", e=D),
                ).then_inc(s_tf, 16)

            # chunk output writes: partition b holds rows b*A..b*A+A-1
            for m in range(NCH):
                if m < _k:
                    s.wait_ge(s_gf[m % H32], 16 * (m // H32 + 1))
                else:
                    s.wait_ge(s_ct[(m - _k) % H16], (m - _k) // H16 + 1)
                s.dma_start(
                    out[m * CHUNK:(m + 1) * CHUNK, :].rearrange("(b x) e -> b (x e)", x=A),
                    g32[:, g32s(m)],
                ).then_inc(s_w[m % H32], 16)
            for h in range(H32):
                s.wait_ge(s_w[h], 16 * (NCH // H32))

        @block.gpsimd
        def _(g: bass.BassGpSimd):
            g.load_library(mlp)
            g.wait_ge(s_ids, 16)
            # leading f32 chunks straight from tbl_cat (idx cols hold
            # host-composed flat_map[ids]; no table-build dependency)
            for m in range(_k):
                if m >= H32:
                    g.wait_ge(s_w[m % H32], 16 * (m // H32))
                g.dma_gather(
                    g32[:, g32s(m)].rearrange("p (n e) -> p n e", e=D),
                    tbl_cat[:, :], ids16[:, idxs(m)],
                    CHUNK, CHUNK, D, queue_num=(m + 2) % _nq,
                ).then_inc(s_gf[m % H32], 16)
            # remap slices: tbl_fin16 row g = f16(tbl_cat[flat_map[g]])
            g.wait_ge(s_fm, 16)
            for i in range(2):
                g.dma_gather(
                    rdst[:, i * JH * D:(i + 1) * JH * D].rearrange("p (j e) -> p j e", e=D),
                    tbl_cat[:, :], fm16[:, i * HALF:(i + 1) * HALF],
                    RSPLIT, RSPLIT, D, queue_num=i % _nq,
                ).then_inc(s_gr[i], 16)
            # f16 main gathers
            g.wait_ge(s_tf, 32)
            for m in range(_k, NCH):
                if m - _k >= H16:
                    g.wait_ge(s_ct[(m - _k) % H16], (m - _k) // H16)
                g.dma_gather(
                    g16[:, g16s(m)].rearrange("p (n e) -> p n e", e=D),
                    tbl_fin16[:, :], ids16[:, idxs(m)],
                    CHUNK, CHUNK, D, queue_num=m % _nq,
                ).then_inc(s_g16[(m - _k) % H16], 16)

    nc.compile()
    return nc


_NC_CACHE: list = [None]


def _get_nc() -> bacc.Bacc:
    if _NC_CACHE[0] is None:
        _NC_CACHE[0] = build_nc()
    return _NC_CACHE[0]


TAB_ORDER = [
    "special_tab", "event_tab", "time_tab", "note_tab", "vel_tab", "prog_tab",
    "local_tab", "ccnum_tab", "ccval_tab", "progval_tab", "dur_tab",
]


def make_in_maps(**inputs) -> list[dict]:
    ids_full = np.asarray(inputs["input_ids"], dtype=np.int32)
    flat_map = np.asarray(inputs["flat_map"], dtype=np.int32)
    tbl_cat = np.concatenate(
        [np.asarray(inputs[name], dtype=np.float32) for name in TAB_ORDER]
        + [np.asarray(inputs["unknown_embed"], dtype=np.float32)[None, :]],
        axis=0)
    # flat_map, padded to RIDX, wrapped [q, s] = fm[s*16+q], replicated x8
    fmp = np.zeros(RIDX, dtype=np.int16)
    fmp[:VOCAB] = flat_map.astype(np.int16)
    fm16w = np.ascontiguousarray(np.tile(fmp.reshape(-1, 16).T, (8, 1)))
    identity_fm = bool(np.array_equal(flat_map, np.arange(VOCAB)))
    shared = {
        "fm16w": fm16w,
        "tbl_cat": np.ascontiguousarray(tbl_cat),
    }
    in_maps = []
    for c in range(NCORES):
        ids_c = ids_full[c * BPC:(c + 1) * BPC, :].reshape(-1)
        if not identity_fm:
            # leading K_F32 chunks read tbl_cat directly and need composed idxs
            ids_c = ids_c.copy()
            ids_c[:K_F32 * CHUNK] = flat_map[ids_c[:K_F32 * CHUNK]]
        # wrapped idx layout: idsw[q, c*64 + a*8 + cc] = ids[c*1024+cc*128+q*8+a]
        idsw = ids_c.reshape(NCH, CC, 16, A).transpose(2, 0, 3, 1).reshape(16, -1)
        m = dict(shared)
        m["ids16w"] = np.ascontiguousarray(np.tile(idsw.astype(np.int16), (8, 1)))
        in_maps.append(m)
    return in_maps


def kernel(**inputs) -> np.ndarray:
    nc = _get_nc()
    in_maps = make_in_maps(**inputs)
    res = run_bass_kernel_spmd(nc, in_maps, list(range(NCORES)))
    outs = [res.results[c]["out"] for c in range(NCORES)]
    return np.concatenate(outs, axis=0).reshape(B, S, D)


def kernel_traced(**inputs):
    """Like kernel() but with NTFF profiling; returns (output, BassKernelResults)."""
    nc = _get_nc()
    in_maps = make_in_maps(**inputs)
    res = run_bass_kernel_spmd(nc, in_maps, list(range(NCORES)), trace=True)
    outs = [res.results[c]["out"] for c in range(NCORES)]
    return np.concatenate(outs, axis=0).reshape(B, S, D), res
